# revision 1
# baseline (speedup 1.0000x reference)
"""MultiHeadSelfAttention + residual + LayerNorm on 8 TRN2 NeuronCores.

Sharding: 2 cores per batch element (B=4), heads split 8/8 within the pair
(tensor parallel). Each core: QKV for its heads over the full sequence,
attention (packed head pairs, [V|ones] softmax-denominator fold, f32r
matmuls), row-sharded output projection partial, pairwise AllReduce per
512-row block (overlapped with the next chunk's compute), residual +
LayerNorm per block.

Softmax denominators are broadcast across partitions via a DRAM bounce
(SBUF-source partition-broadcast DMA is illegal, and gpsimd
partition_broadcast would serialize behind the blocking collectives on the
gpsimd queue). Reciprocals run on a [64, 8] scatter so all DVE lanes work.

Self-contained: shapes/sharding hardcoded; builds and caches the NEFF on
first call. Output y is the full batch row range; the host slices each
core's half.
"""
import numpy as np

import concourse.bass as bass
import concourse.tile as tile
from concourse import bacc, mybir
from concourse.bass_utils import run_bass_kernel_spmd
from concourse.masks import make_identity

F32 = mybir.dt.float32
F32R = mybir.dt.float32r

B, S, D, H, DEPTH = 4, 2048, 1024, 16, 64
HL = 8            # heads per core
EL = 512          # local e width (HL * DEPTH)
CT = 8            # c tiles (D / 128)
ST = 16           # s tiles (S / 128)
SC = 4            # s chunks (S / 512)
ET = 4            # local e tiles (EL / 128)
EPS = 1e-6
RG = [[0, 1], [2, 3], [4, 5], [6, 7]]

_CACHE = {}
_LAST_IN_MAPS = None


def _build():
    nc = bacc.Bacc("TRN2", target_bir_lowering=False, debug=False, num_devices=8)

    x_in = nc.dram_tensor("x", [S, D], F32R, kind="ExternalInput")
    wq_in = nc.dram_tensor("wq", [D, EL], F32R, kind="ExternalInput")
    wk_in = nc.dram_tensor("wk", [D, EL], F32R, kind="ExternalInput")
    wv_in = nc.dram_tensor("wv", [D, EL], F32R, kind="ExternalInput")
    wo_in = nc.dram_tensor("wo", [EL, D], F32R, kind="ExternalInput")
    bqk_in = nc.dram_tensor("bqk", [128, 2 * ET], F32, kind="ExternalInput")
    bv_in = nc.dram_tensor("bv", [1, EL], F32, kind="ExternalInput")
    bo_in = nc.dram_tensor("bo", [1, D], F32, kind="ExternalInput")
    gamma_in = nc.dram_tensor("gamma", [1, D], F32, kind="ExternalInput")
    beta_in = nc.dram_tensor("beta", [1, D], F32, kind="ExternalInput")
    y_out = nc.dram_tensor("y", [S, D], F32, kind="ExternalOutput")

    with tile.TileContext(nc) as tc:
        with tc.tile_pool(name="const", bufs=1) as const, \
             tc.tile_pool(name="dram", bufs=1, space="DRAM") as dram:

            ident_f = const.tile([128, 128], F32)
            make_identity(nc, ident_f[:])
            ident = const.tile([128, 128], F32R)
            nc.vector.tensor_copy(ident[:], ident_f[:])
            ones1 = const.tile([128, 1], F32)
            nc.gpsimd.memset(ones1[:], 1.0)
            eps_sb = const.tile([128, 1], F32)
            nc.gpsimd.memset(eps_sb[:], EPS)

            bqk_sb = const.tile([128, 2 * ET], F32)
            nc.sync.dma_start(bqk_sb[:], bqk_in.ap()[:])
            bv_bc = const.tile([128, EL], F32)
            nc.sync.dma_start(bv_bc[:], bv_in.ap().to_broadcast((128, EL)))

            y_part = [dram.tile([1024, D], F32, name=f"y_part{i}") for i in range(2)]
            ar_out = [dram.tile([1024, D], F32, name=f"ar_out{i}") for i in range(2)]
            den_d = dram.tile([2 * ET * 2, 1, 1024], F32)
            rec_d = dram.tile([2 * ET * 2, 64, 16], F32)

            with tc.tile_pool(name="qkv", bufs=1) as qkvp:
                kt = qkvp.tile([128, ET, S], F32R)                 # K^T [e, s]
                qt = qkvp.tile([128, ET, S], F32R)                 # Q^T [e, s]
                vt = qkvp.tile([128, ST, HL, DEPTH + 1], F32R)     # V natural + ones
                nc.vector.tensor_copy(vt[:, :, :, DEPTH:DEPTH + 1],
                                      ones1[:].to_broadcast((128, ST, HL, 1)))

                # ---- phase A: transpose X per chunk; project Q, K, V ----
                with tc.tile_pool(name="xnA", bufs=3) as xnA, \
                     tc.tile_pool(name="xtA", bufs=2) as xtA, \
                     tc.tile_pool(name="w3", bufs=1) as w3, \
                     tc.tile_pool(name="tpA", bufs=4, space="PSUM") as tpA, \
                     tc.tile_pool(name="psA", bufs=4, space="PSUM") as psA:
                    wsb = {}
                    for nm, wdram in (("q", wq_in), ("k", wk_in), ("v", wv_in)):
                        wsb[nm] = w3.tile([128, CT, EL], F32R, name=f"w{nm}")
                        for ci in range(CT):
                            nc.sync.dma_start(wsb[nm][:, ci, :],
                                              wdram.ap()[128 * ci:128 * (ci + 1), :])
                    for sc in range(SC):
                        cs = slice(512 * sc, 512 * (sc + 1))
                        xt_c = xtA.tile([128, CT, 512], F32R, name="xt_c", tag="xt_c")
                        for sl in range(4):
                            si = 4 * sc + sl
                            xn = xnA.tile([128, D], F32R, name="xn", tag="xn")
                            nc.sync.dma_start(xn[:], x_in.ap()[128 * si:128 * (si + 1), :])
                            for ci in range(CT):
                                tp = tpA.tile([128, 128], F32R, name="tp", tag="tp")
                                nc.tensor.transpose(tp[:], xn[:, 128 * ci:128 * (ci + 1)],
                                                    ident[:])
                                nc.vector.tensor_copy(xt_c[:, ci, 128 * sl:128 * (sl + 1)],
                                                      tp[:])
                        for dst, wname, bcol in ((qt, "q", 0), (kt, "k", ET)):
                            for j in range(ET):
                                ps = psA.tile([128, 512], F32, name="pqk", tag="pqk")
                                for ci in range(CT):
                                    nc.tensor.matmul(
                                        ps[:], wsb[wname][:, ci, 128 * j:128 * (j + 1)],
                                        xt_c[:, ci, :], start=(ci == 0), stop=(ci == CT - 1))
                                nc.vector.tensor_scalar_add(
                                    dst[:, j, cs], ps[:], bqk_sb[:, bcol + j:bcol + j + 1])
                        for sl in range(4):
                            si = 4 * sc + sl
                            ps = psA.tile([128, 512], F32, name="pv", tag="pqk")
                            for ci in range(CT):
                                nc.tensor.matmul(
                                    ps[:], xt_c[:, ci, 128 * sl:128 * (sl + 1)],
                                    wsb["v"][:, ci, :], start=(ci == 0), stop=(ci == CT - 1))
                            nc.vector.tensor_add(
                                vt[:, si, :, 0:DEPTH],
                                ps[:].rearrange("p (h e) -> p h e", h=HL),
                                bv_bc[:].rearrange("p (h e) -> p h e", h=HL))

                # ---- phase B: attention per q-chunk + chunk projection + AR ----
                with tc.tile_pool(name="wo", bufs=1) as wop, \
                     tc.tile_pool(name="atc", bufs=1) as atcp, \
                     tc.tile_pool(name="ep3", bufs=2) as ep3, \
                     tc.tile_pool(name="psb", bufs=2) as psb, \
                     tc.tile_pool(name="ysb", bufs=2) as ysb, \
                     tc.tile_pool(name="sps", bufs=2, space="PSUM") as sps, \
                     tc.tile_pool(name="aps", bufs=1, space="PSUM") as aps, \
                     tc.tile_pool(name="ps4", bufs=2, space="PSUM") as ps4:
                    wo_sb = wop.tile([128, ET, D], F32R)
                    for j in range(ET):
                        nc.sync.dma_start(wo_sb[:, j, :], wo_in.ap()[128 * j:128 * (j + 1), :])
                    for qc in range(2):
                        a_t = atcp.tile([128, ET, 1024], F32R, name="a_t", tag="a_t")
                        for j in range(ET):
                            accs = [aps.tile([DEPTH + 1, 1024], F32, name=f"acc{h}", tag=f"acc{h}")
                                    for h in range(2)]
                            def emit_pv(kp, pp):
                                for half in range(2):
                                    for h01 in range(2):
                                        nc.tensor.matmul(
                                            accs[h01][:, 512 * half:512 * (half + 1)],
                                            vt[:, kp, 2 * j + h01, :],
                                            pp[(h01, half)][:],
                                            start=(kp == 0), stop=(kp == ST - 1))

                            p_prev = None
                            for kti in range(ST):
                                ks = slice(128 * kti, 128 * (kti + 1))
                                # pv of the previous tile first so PE never
                                # head-of-line blocks on this tile's exp
                                if p_prev is not None:
                                    emit_pv(kti - 1, p_prev)
                                # interleave head A/B so the K=64 score matmuls
                                # pack into disjoint PE row groups
                                s_q = {}
                                for half in range(2):
                                    qs = slice(1024 * qc + 512 * half,
                                               1024 * qc + 512 * (half + 1))
                                    for h01 in range(2):
                                        rows = slice(64 * h01, 64 * (h01 + 1))
                                        s_ps = sps.tile([128, 512], F32,
                                                        name=f"s{h01}_{half}",
                                                        tag=f"s{h01}")
                                        nc.tensor.matmul(s_ps[:], kt[rows, j, ks],
                                                         qt[rows, j, qs],
                                                         start=True, stop=True)
                                        s_q[(h01, half)] = s_ps
                                ps_quad = {}
                                for half in range(2):
                                    for h01 in range(2):
                                        p = psb.tile([128, 512], F32R,
                                                     name=f"p{h01}_{half}",
                                                     tag=f"p{h01}_{half}")
                                        nc.scalar.activation(p[:], s_q[(h01, half)][:],
                                                             mybir.ActivationFunctionType.Exp,
                                                             scale=0.125)
                                        ps_quad[(h01, half)] = p
                                p_prev = ps_quad
                            emit_pv(ST - 1, p_prev)
                            for h01 in range(2):
                                idx = (qc * ET + j) * 2 + h01
                                acc_sb = ep3.tile([DEPTH + 1, 1024], F32, name="acc_sb", tag="acc_sb")
                                nc.vector.tensor_copy(acc_sb[:], accs[h01][:])
                                nc.sync.dma_start(den_d[idx],
                                                  acc_sb[DEPTH:DEPTH + 1, :])
                                rin = ep3.tile([64, 16], F32, name="rin", tag="rin")
                                nc.sync.dma_start(rin[:], den_d[idx].rearrange("a (p f) -> (a p) f", p=64))
                                nc.vector.reciprocal(rin[:], rin[:])
                                nc.sync.dma_start(rec_d[idx], rin[:])
                                rbc = ep3.tile([64, 1024], F32, name="rbc", tag="rbc")
                                rsrc = rec_d[idx]
                                nc.sync.dma_start(
                                    rbc[:],
                                    bass.AP(tensor=rsrc.tensor, offset=rsrc.offset,
                                            ap=[[0, 64], [1, 1024]]))
                                if h01 == 0:
                                    nc.vector.tensor_mul(a_t[0:64, j, :],
                                                         acc_sb[0:DEPTH, :], rbc[:])
                                else:
                                    nrm = ep3.tile([64, 1024], F32R, name="nrm", tag="nrm")
                                    nc.vector.tensor_mul(nrm[:], acc_sb[0:DEPTH, :], rbc[:])
                                    nc.sync.dma_start(a_t[64:128, j, :], nrm[:])
                        # chunk projection: rows 1024*qc .. 1024*(qc+1)
                        for stl in range(8):
                            ss = slice(128 * stl, 128 * (stl + 1))
                            for mh in range(2):
                                ms = slice(512 * mh, 512 * (mh + 1))
                                ps = aps.tile([128, 512], F32, name="py",
                                              tag=f"acc{stl % 2}")
                                for j in range(ET):
                                    nc.tensor.matmul(ps[:], a_t[:, j, ss], wo_sb[:, j, ms],
                                                     start=(j == 0), stop=(j == ET - 1))
                                y_sb = ysb.tile([128, 512], F32, name="y_sb", tag="y_sb")
                                nc.vector.tensor_copy(y_sb[:], ps[:])
                                nc.sync.dma_start(y_part[qc][ss, ms], y_sb[:])
                        nc.gpsimd.collective_compute(
                            "AllReduce", mybir.AluOpType.add,
                            replica_groups=RG,
                            ins=[y_part[qc].opt()], outs=[ar_out[qc].opt()])

            # ---- phase C: residual + LayerNorm per block ----
            with tc.tile_pool(name="lnc", bufs=1) as lnc, \
                 tc.tile_pool(name="ln", bufs=3) as ln:
                bo_bc = lnc.tile([128, D], F32)
                nc.sync.dma_start(bo_bc[:], bo_in.ap().to_broadcast((128, D)))
                gam_bc = lnc.tile([128, D], F32)
                nc.sync.dma_start(gam_bc[:], gamma_in.ap().to_broadcast((128, D)))
                bet_bc = lnc.tile([128, D], F32)
                nc.sync.dma_start(bet_bc[:], beta_in.ap().to_broadcast((128, D)))
                for blk in range(2):
                    for rt in range(8):
                        rs = slice(128 * rt, 128 * (rt + 1))
                        grow = slice(1024 * blk + 128 * rt, 1024 * blk + 128 * (rt + 1))
                        t = ln.tile([128, D], F32, name="t", tag="t")
                        nc.sync.dma_start(t[:], ar_out[blk][rs, :])
                        r = ln.tile([128, D], F32, name="r", tag="r")
                        nc.sync.dma_start(r[:], x_in.ap()[grow, :].bitcast(F32))
                        nc.vector.tensor_add(t[:], t[:], r[:])
                        nc.vector.tensor_add(t[:], t[:], bo_bc[:])
                        stats = ln.tile([128, 2, 6], F32, name="stats", tag="stats")
                        tv = t[:].rearrange("p (a b) -> p a b", a=2)
                        for sub in range(2):
                            nc.vector.bn_stats(stats[:, sub, :], tv[:, sub, :])
                        mv = ln.tile([128, 2], F32, name="mv", tag="mv")
                        nc.vector.bn_aggr(mv[:], stats[:])
                        std = ln.tile([128, 1], F32, name="std", tag="std")
                        nc.scalar.activation(std[:], mv[:, 1:2],
                                             mybir.ActivationFunctionType.Sqrt,
                                             bias=eps_sb[:], scale=1.0)
                        nc.vector.reciprocal(std[:], std[:])
                        o = ln.tile([128, D], F32, name="o", tag="o")
                        nc.vector.tensor_scalar(
                            o[:], t[:], mv[:, 0:1], std[:],
                            mybir.AluOpType.subtract, mybir.AluOpType.mult)
                        nc.vector.tensor_mul(o[:], o[:], gam_bc[:])
                        nc.vector.tensor_add(o[:], o[:], bet_bc[:])
                        nc.sync.dma_start(y_out.ap()[grow, :], o[:])

    nc.compile()
    return nc


def kernel(inputs, Wq, bq, Wk, bk, Wv, bv, Wo, bo, gamma, beta):
    if "nc" not in _CACHE:
        _CACHE["nc"] = _build()
    nc = _CACHE["nc"]

    inputs = np.ascontiguousarray(np.asarray(inputs, dtype=np.float32))
    Wq = np.asarray(Wq, np.float32); Wk = np.asarray(Wk, np.float32)
    Wv = np.asarray(Wv, np.float32); Wo = np.asarray(Wo, np.float32)
    bq = np.asarray(bq, np.float32); bk = np.asarray(bk, np.float32)
    bv = np.asarray(bv, np.float32); bo = np.asarray(bo, np.float32)
    gamma = np.asarray(gamma, np.float32); beta = np.asarray(beta, np.float32)

    in_maps = []
    for c in range(8):
        b, hf = c // 2, c % 2
        es = slice(EL * hf, EL * (hf + 1))
        bqk = np.concatenate([bq[es].reshape(ET, 128).T, bk[es].reshape(ET, 128).T],
                             axis=1)
        in_maps.append({
            "x": inputs[b],
            "wq": np.ascontiguousarray(Wq[:, es]),
            "wk": np.ascontiguousarray(Wk[:, es]),
            "wv": np.ascontiguousarray(Wv[:, es]),
            "wo": np.ascontiguousarray(Wo[es, :]),
            "bqk": np.ascontiguousarray(bqk),
            "bv": bv[es].reshape(1, EL).copy(),
            "bo": bo.reshape(1, D).copy(),
            "gamma": gamma.reshape(1, D).copy(),
            "beta": beta.reshape(1, D).copy(),
        })

    global _LAST_IN_MAPS
    _LAST_IN_MAPS = in_maps
    res = run_bass_kernel_spmd(nc, in_maps, core_ids=list(range(8)))

    out = np.empty((B, S, D), dtype=np.float32)
    for c in range(8):
        b, hf = c // 2, c % 2
        out[b, 1024 * hf:1024 * (hf + 1)] = res.results[c]["y"][1024 * hf:1024 * (hf + 1)]
    return out



# revision 2
# speedup vs baseline: 1.3194x; 1.3194x over previous
"""MultiHeadSelfAttention + residual + LayerNorm on 8 TRN2 NeuronCores.

Sharding: 2 cores per batch element (B=4), heads split 8/8 within the pair
(tensor parallel). Each core: QKV for its heads over the full sequence,
attention in 512-query chunks (packed head pairs, [V|ones] softmax-denominator
fold, f32r matmuls, one batched exp per key-tile covering both heads),
row-sharded output projection, fine-grained pairwise AllReduce per 256-row
block overlapped with later chunks' compute, residual + LayerNorm per block
also inside the loop so only the last small block's AR+LN is exposed.

Softmax denominators are broadcast across partitions via a DRAM bounce
(SBUF-source partition-broadcast DMA is illegal, and gpsimd
partition_broadcast would serialize behind the blocking collectives on the
gpsimd queue). Reciprocals run on a [64, 8] scatter so all DVE lanes work.

Self-contained: shapes/sharding hardcoded; builds and caches the NEFF on
first call. Output y is the full batch row range; the host slices each
core's half.
"""
import numpy as np

import concourse.bass as bass
import concourse.tile as tile
from concourse import bacc, mybir
from concourse.bass_utils import run_bass_kernel_spmd
from concourse.masks import make_identity

F32 = mybir.dt.float32
F32R = mybir.dt.float32r

B, S, D, H, DEPTH = 4, 2048, 1024, 16, 64
HL = 8            # heads per core
EL = 512          # local e width (HL * DEPTH)
CT = 8            # c tiles (D / 128)
ST = 16           # s tiles (S / 128)
SC = 4            # s chunks (S / 512)
ET = 4            # local e tiles (EL / 128)
QC = 4            # query chunks in phase B (S / 512)
EPS = 1e-6
RG = [[0, 1], [2, 3], [4, 5], [6, 7]]

_CACHE = {}
_LAST_IN_MAPS = None


def _build():
    nc = bacc.Bacc("TRN2", target_bir_lowering=False, debug=False, num_devices=8)

    x_in = nc.dram_tensor("x", [S, D], F32R, kind="ExternalInput")
    wq_in = nc.dram_tensor("wq", [D, EL], F32R, kind="ExternalInput")
    wk_in = nc.dram_tensor("wk", [D, EL], F32R, kind="ExternalInput")
    wv_in = nc.dram_tensor("wv", [D, EL], F32R, kind="ExternalInput")
    wo_in = nc.dram_tensor("wo", [EL, D], F32R, kind="ExternalInput")
    bqk_in = nc.dram_tensor("bqk", [128, 2 * ET], F32, kind="ExternalInput")
    bv_in = nc.dram_tensor("bv", [1, EL], F32, kind="ExternalInput")
    bo_in = nc.dram_tensor("bo", [1, D], F32, kind="ExternalInput")
    gamma_in = nc.dram_tensor("gamma", [1, D], F32, kind="ExternalInput")
    beta_in = nc.dram_tensor("beta", [1, D], F32, kind="ExternalInput")
    y_out = nc.dram_tensor("y", [S, D], F32, kind="ExternalOutput")

    with tile.TileContext(nc) as tc:
        with tc.tile_pool(name="const", bufs=1) as const, \
             tc.tile_pool(name="dram", bufs=1, space="DRAM") as dram:

            ident_f = const.tile([128, 128], F32)
            make_identity(nc, ident_f[:])
            ident = const.tile([128, 128], F32R)
            nc.vector.tensor_copy(ident[:], ident_f[:])
            ones1 = const.tile([128, 1], F32)
            nc.gpsimd.memset(ones1[:], 1.0)
            eps_sb = const.tile([128, 1], F32)
            nc.gpsimd.memset(eps_sb[:], EPS)

            bqk_sb = const.tile([128, 2 * ET], F32)
            nc.sync.dma_start(bqk_sb[:], bqk_in.ap()[:])
            bv_bc = const.tile([128, EL], F32)
            nc.sync.dma_start(bv_bc[:], bv_in.ap().to_broadcast((128, EL)))

            # 8 AllReduce blocks of 256 rows each
            y_part = [dram.tile([256, D], F32, name=f"y_part{i}") for i in range(8)]
            ar_out = [dram.tile([256, D], F32, name=f"ar_out{i}") for i in range(8)]
            den_d = dram.tile([QC * ET * 2, 1, 512], F32)
            rec_d = dram.tile([QC * ET * 2, 64, 8], F32)

            with tc.tile_pool(name="qkv", bufs=1) as qkvp:
                kt = qkvp.tile([128, ET, S], F32R)                 # K^T [e, s]
                qt = qkvp.tile([128, ET, S], F32R)                 # Q^T [e, s]
                vt = qkvp.tile([128, ST, HL, DEPTH + 1], F32R)     # V natural + ones
                nc.vector.tensor_copy(vt[:, :, :, DEPTH:DEPTH + 1],
                                      ones1[:].to_broadcast((128, ST, HL, 1)))

                # ---- phase A: transpose X per chunk; project Q, K, V ----
                with tc.tile_pool(name="xnA", bufs=3) as xnA, \
                     tc.tile_pool(name="xtA", bufs=2) as xtA, \
                     tc.tile_pool(name="w3", bufs=1) as w3, \
                     tc.tile_pool(name="tpA", bufs=4, space="PSUM") as tpA, \
                     tc.tile_pool(name="psA", bufs=4, space="PSUM") as psA:
                    wsb = {}
                    for nm, wdram in (("q", wq_in), ("k", wk_in), ("v", wv_in)):
                        wsb[nm] = w3.tile([128, CT, EL], F32R, name=f"w{nm}")
                        for ci in range(CT):
                            nc.sync.dma_start(wsb[nm][:, ci, :],
                                              wdram.ap()[128 * ci:128 * (ci + 1), :])
                    for sc in range(SC):
                        cs = slice(512 * sc, 512 * (sc + 1))
                        xt_c = xtA.tile([128, CT, 512], F32R, name="xt_c", tag="xt_c")
                        for sl in range(4):
                            si = 4 * sc + sl
                            xn = xnA.tile([128, D], F32R, name="xn", tag="xn")
                            nc.sync.dma_start(xn[:], x_in.ap()[128 * si:128 * (si + 1), :])
                            for ci in range(CT):
                                tp = tpA.tile([128, 128], F32R, name="tp", tag="tp")
                                nc.tensor.transpose(tp[:], xn[:, 128 * ci:128 * (ci + 1)],
                                                    ident[:])
                                nc.vector.tensor_copy(xt_c[:, ci, 128 * sl:128 * (sl + 1)],
                                                      tp[:])
                        for dst, wname, bcol in ((qt, "q", 0), (kt, "k", ET)):
                            for j in range(ET):
                                ps = psA.tile([128, 512], F32, name="pqk", tag="pqk")
                                for ci in range(CT):
                                    nc.tensor.matmul(
                                        ps[:], wsb[wname][:, ci, 128 * j:128 * (j + 1)],
                                        xt_c[:, ci, :], start=(ci == 0), stop=(ci == CT - 1))
                                nc.vector.tensor_scalar_add(
                                    dst[:, j, cs], ps[:], bqk_sb[:, bcol + j:bcol + j + 1])
                        for sl in range(4):
                            si = 4 * sc + sl
                            ps = psA.tile([128, 512], F32, name="pv", tag="pqk")
                            for ci in range(CT):
                                nc.tensor.matmul(
                                    ps[:], xt_c[:, ci, 128 * sl:128 * (sl + 1)],
                                    wsb["v"][:, ci, :], start=(ci == 0), stop=(ci == CT - 1))
                            nc.vector.tensor_add(
                                vt[:, si, :, 0:DEPTH],
                                ps[:].rearrange("p (h e) -> p h e", h=HL),
                                bv_bc[:].rearrange("p (h e) -> p h e", h=HL))

                # ---- phase B: attention per 512-q chunk + projection + AR + LN ----
                with tc.tile_pool(name="wo", bufs=1) as wop, \
                     tc.tile_pool(name="lnc", bufs=1) as lnc, \
                     tc.tile_pool(name="atc", bufs=2) as atcp, \
                     tc.tile_pool(name="ep3", bufs=2) as ep3, \
                     tc.tile_pool(name="psb", bufs=2) as psb, \
                     tc.tile_pool(name="ysb", bufs=2) as ysb, \
                     tc.tile_pool(name="ln", bufs=2) as ln, \
                     tc.tile_pool(name="sps", bufs=2, space="PSUM") as sps, \
                     tc.tile_pool(name="aps", bufs=1, space="PSUM") as aps, \
                     tc.tile_pool(name="psO", bufs=1, space="PSUM") as psO:
                    wo_sb = wop.tile([128, ET, D], F32R)
                    for j in range(ET):
                        nc.sync.dma_start(wo_sb[:, j, :], wo_in.ap()[128 * j:128 * (j + 1), :])
                    bo_bc = lnc.tile([128, D], F32)
                    nc.sync.dma_start(bo_bc[:], bo_in.ap().to_broadcast((128, D)))
                    gam_bc = lnc.tile([128, D], F32)
                    nc.sync.dma_start(gam_bc[:], gamma_in.ap().to_broadcast((128, D)))
                    bet_bc = lnc.tile([128, D], F32)
                    nc.sync.dma_start(bet_bc[:], beta_in.ap().to_broadcast((128, D)))

                    for qc in range(QC):
                        qs = slice(512 * qc, 512 * (qc + 1))
                        a_t = atcp.tile([128, ET, 512], F32R, name="a_t", tag="a_t")
                        for j in range(ET):
                            accs = [aps.tile([DEPTH + 1, 512], F32, name=f"acc{h}",
                                             tag=f"acc{h}")
                                    for h in range(2)]

                            def emit_pv(kp, pp):
                                for h01 in range(2):
                                    nc.tensor.matmul(
                                        accs[h01][:],
                                        vt[:, kp, 2 * j + h01, :],
                                        pp[:, 512 * h01:512 * (h01 + 1)],
                                        start=(kp == 0), stop=(kp == ST - 1))

                            p_prev = None
                            for kti in range(ST):
                                ks = slice(128 * kti, 128 * (kti + 1))
                                # pv of the previous tile first so PE never
                                # head-of-line blocks on this tile's exp
                                if p_prev is not None:
                                    emit_pv(kti - 1, p_prev)
                                # both heads' scores into one 2-bank PSUM tile
                                # so a single batched exp covers them
                                sp = sps.tile([128, 1024], F32, name="sp", tag="sp")
                                for h01 in range(2):
                                    rows = slice(64 * h01, 64 * (h01 + 1))
                                    nc.tensor.matmul(sp[:, 512 * h01:512 * (h01 + 1)],
                                                     kt[rows, j, ks], qt[rows, j, qs],
                                                     start=True, stop=True)
                                pp = psb.tile([128, 1024], F32R, name="pp", tag="pp")
                                nc.scalar.activation(pp[:], sp[:],
                                                     mybir.ActivationFunctionType.Exp,
                                                     scale=0.125)
                                p_prev = pp
                            emit_pv(ST - 1, p_prev)
                            for h01 in range(2):
                                idx = (qc * ET + j) * 2 + h01
                                acc_sb = ep3.tile([DEPTH + 1, 512], F32, name="acc_sb",
                                                  tag="acc_sb")
                                nc.vector.tensor_copy(acc_sb[:], accs[h01][:])
                                nc.sync.dma_start(den_d[idx],
                                                  acc_sb[DEPTH:DEPTH + 1, :])
                                rin = ep3.tile([64, 8], F32, name="rin", tag="rin")
                                nc.sync.dma_start(rin[:], den_d[idx].rearrange(
                                    "a (p f) -> (a p) f", p=64))
                                nc.vector.reciprocal(rin[:], rin[:])
                                nc.sync.dma_start(rec_d[idx], rin[:])
                                rbc = ep3.tile([64, 512], F32, name="rbc", tag="rbc")
                                rsrc = rec_d[idx]
                                nc.sync.dma_start(
                                    rbc[:],
                                    bass.AP(tensor=rsrc.tensor, offset=rsrc.offset,
                                            ap=[[0, 64], [1, 512]]))
                                if h01 == 0:
                                    nc.vector.tensor_mul(a_t[0:64, j, :],
                                                         acc_sb[0:DEPTH, :], rbc[:])
                                else:
                                    nrm = ep3.tile([64, 512], F32R, name="nrm", tag="nrm")
                                    nc.vector.tensor_mul(nrm[:], acc_sb[0:DEPTH, :], rbc[:])
                                    nc.sync.dma_start(a_t[64:128, j, :], nrm[:])
                        # projection + AR + LN per 256-row block (2 per chunk)
                        for half in range(2):
                            blk = 2 * qc + half
                            for stl in range(2):
                                rloc = slice(256 * half + 128 * stl,
                                             256 * half + 128 * (stl + 1))
                                for mh in range(2):
                                    ms = slice(512 * mh, 512 * (mh + 1))
                                    ps = psO.tile([128, 512], F32, name="py",
                                                  tag=f"po{stl}")
                                    for j in range(ET):
                                        nc.tensor.matmul(ps[:], a_t[:, j, rloc],
                                                         wo_sb[:, j, ms],
                                                         start=(j == 0),
                                                         stop=(j == ET - 1))
                                    y_sb = ysb.tile([128, 512], F32, name="y_sb",
                                                    tag="y_sb")
                                    nc.vector.tensor_copy(y_sb[:], ps[:])
                                    nc.sync.dma_start(
                                        y_part[blk][128 * stl:128 * (stl + 1), ms],
                                        y_sb[:])
                            nc.gpsimd.collective_compute(
                                "AllReduce", mybir.AluOpType.add,
                                replica_groups=RG,
                                ins=[y_part[blk].opt()], outs=[ar_out[blk].opt()])
                            # residual + LayerNorm for this block, overlapped
                            for rt in range(2):
                                grow = slice(256 * blk + 128 * rt,
                                             256 * blk + 128 * (rt + 1))
                                t = ln.tile([128, D], F32, name="t", tag="t")
                                nc.sync.dma_start(
                                    t[:], ar_out[blk][128 * rt:128 * (rt + 1), :])
                                r = ln.tile([128, D], F32, name="r", tag="r")
                                nc.sync.dma_start(r[:], x_in.ap()[grow, :].bitcast(F32))
                                nc.vector.tensor_add(t[:], t[:], r[:])
                                nc.vector.tensor_add(t[:], t[:], bo_bc[:])
                                stats = ln.tile([128, 2, 6], F32, name="stats",
                                                tag="stats")
                                tv = t[:].rearrange("p (a b) -> p a b", a=2)
                                for sub in range(2):
                                    nc.vector.bn_stats(stats[:, sub, :], tv[:, sub, :])
                                mv = ln.tile([128, 2], F32, name="mv", tag="mv")
                                nc.vector.bn_aggr(mv[:], stats[:])
                                std = ln.tile([128, 1], F32, name="std", tag="std")
                                nc.scalar.activation(std[:], mv[:, 1:2],
                                                     mybir.ActivationFunctionType.Sqrt,
                                                     bias=eps_sb[:], scale=1.0)
                                nc.vector.reciprocal(std[:], std[:])
                                o = ln.tile([128, D], F32, name="o", tag="o")
                                nc.vector.tensor_scalar(
                                    o[:], t[:], mv[:, 0:1], std[:],
                                    mybir.AluOpType.subtract, mybir.AluOpType.mult)
                                nc.vector.tensor_mul(o[:], o[:], gam_bc[:])
                                nc.vector.tensor_add(o[:], o[:], bet_bc[:])
                                nc.sync.dma_start(y_out.ap()[grow, :], o[:])

    nc.compile()
    return nc


def kernel(inputs, Wq, bq, Wk, bk, Wv, bv, Wo, bo, gamma, beta):
    if "nc" not in _CACHE:
        _CACHE["nc"] = _build()
    nc = _CACHE["nc"]

    inputs = np.ascontiguousarray(np.asarray(inputs, dtype=np.float32))
    Wq = np.asarray(Wq, np.float32); Wk = np.asarray(Wk, np.float32)
    Wv = np.asarray(Wv, np.float32); Wo = np.asarray(Wo, np.float32)
    bq = np.asarray(bq, np.float32); bk = np.asarray(bk, np.float32)
    bv = np.asarray(bv, np.float32); bo = np.asarray(bo, np.float32)
    gamma = np.asarray(gamma, np.float32); beta = np.asarray(beta, np.float32)

    in_maps = []
    for c in range(8):
        b, hf = c // 2, c % 2
        es = slice(EL * hf, EL * (hf + 1))
        bqk = np.concatenate([bq[es].reshape(ET, 128).T, bk[es].reshape(ET, 128).T],
                             axis=1)
        in_maps.append({
            "x": inputs[b],
            "wq": np.ascontiguousarray(Wq[:, es]),
            "wk": np.ascontiguousarray(Wk[:, es]),
            "wv": np.ascontiguousarray(Wv[:, es]),
            "wo": np.ascontiguousarray(Wo[es, :]),
            "bqk": np.ascontiguousarray(bqk),
            "bv": bv[es].reshape(1, EL).copy(),
            "bo": bo.reshape(1, D).copy(),
            "gamma": gamma.reshape(1, D).copy(),
            "beta": beta.reshape(1, D).copy(),
        })

    global _LAST_IN_MAPS
    _LAST_IN_MAPS = in_maps
    res = run_bass_kernel_spmd(nc, in_maps, core_ids=list(range(8)))

    out = np.empty((B, S, D), dtype=np.float32)
    for c in range(8):
        b, hf = c // 2, c % 2
        out[b, 1024 * hf:1024 * (hf + 1)] = res.results[c]["y"][1024 * hf:1024 * (hf + 1)]
    return out
